# revision 5
# baseline (speedup 1.0000x reference)
"""MultiHeadSelfAttention + ALiBi for Trainium2, SPMD over 8 NeuronCores.

Sharding: core c handles batch b = c // 4 and head group g = c % 4
(3 of the 12 heads, grouped so per-head ALiBi band sizes balance).
Each core computes y_partial[b] = ctx(heads_g) @ Wout[rows_g]; the host
sums the 4 partials per batch and adds bout.

Key-side mask compaction: attn_mask zeroes ~half the key positions, and
masked keys contribute exactly 0 to softmax (exp(-1e9)).  The host
gathers the unmasked key positions (1002 / 1034 for the two batches)
and pads to NK=1152 = 9 key-tiles; K and V are projected only for those
rows, nearly halving the S / exp / PV work vs the full 16-tile key
axis.  Padding key columns carry a -1e9 mask bias (or a zero post-exp
factor) so they drop out of softmax.

All attention matmuls run in float32r (12-mantissa-bit fast mode, 4x
the fp32 rate at free-dim >= 256).  f32r operands must come from
rounding producers: DMA qualifies when the DRAM tensor is declared
f32r (host pre-rounds aug rows); Q/K round via their producing DVE
tensor_scalar, P=exp(S) via ACT, ctx via DVE.

ALiBi handling: for blocks where all of a tile's (original) key
positions fall on one side of the whole query chunk, -s|q-k| is linear
and is computed by augmented contraction rows (mask bias, +-s*k split
into 11-bit value+residual, -+s*q likewise).  Blocks straddling the
diagonal for either batch instead run mask-free mask-less S = Q.K
(rows 0:64 only) and apply the true factor multiplicatively AFTER exp:
pt *= expfix, expfix[p,q] = exp(-s|q - o(k_p)|) host-precomputed in
fp16 (0 at padding keys, which also kills them without a mask row).
The multiply runs on GPSIMD (SBUF-only engine - it cannot touch PSUM,
so the old additive PSUM fixup is impossible there).

Engine split keeps everything under the PE's ~60us: Scalar = exp
stream + final 1/denom (one Ln + one Exp(-x) after all stage-2 exps;
DVE reciprocal misbehaves under this runtime and ACT Reciprocal is
blocked in bass); DVE = Q/K/V PSUM->SBUF rounding stashes, ctx/denom
stashes, y copies; GPSIMD = expfix multiplies, reciprocal broadcasts,
normalize multiplies.  QbufL Q-rows copy via SBUF->SBUF DMA.

Band structure and straddle lists depend only on attn_mask, which is
fixed for this problem instance; the Bass program is cached keyed on
that structure.
"""

import math
import os

import numpy as np


def _ensure_concourse():
    try:
        import concourse  # noqa: F401
    except ImportError:
        import sys

        for p in ("/opt/trn_rl_repo", "/root/.axon_site/_ro/trn_rl_repo"):
            if os.path.isdir(p) and p not in sys.path:
                sys.path.insert(0, p)


B, L, D, H, DH = 2, 2048, 768, 12, 64
QC = L // 512  # 4 q-chunks
NH = 3  # heads per core
N_CORES = 8
NK = 1152  # padded compacted key count (9 tiles)
KT = NK // 128
GROUP_SIZE = 2  # exp/S group size in k-tiles (each group: one PSUM st tile)

# ALiBi cutoff distance per head slot (36/slope of the widest head in the
# slot; exp(-36+swing) leak < 1e-8).  Slot 0 heads are effectively
# unbanded.
D_SLOT = [None, 576, 204]

# Head groups balanced by ALiBi band size: each group gets one wide-band,
# one mid-band and one narrow-band head.
HEAD_GROUPS = [[4, 3, 0], [5, 2, 8], [6, 11, 9], [7, 1, 10]]

NEG_MASK = -1.0e9


def alibi_slopes(n_heads: int) -> np.ndarray:
    def slopes_pow2(n):
        start = 2 ** (-(2 ** -(math.log2(n) - 3)))
        return [start * start**i for i in range(n)]

    if math.log2(n_heads).is_integer():
        s = slopes_pow2(n_heads)
    else:
        cp = 2 ** int(math.floor(math.log2(n_heads)))
        s = slopes_pow2(cp) + slopes_pow2(2 * cp)[0::2][: n_heads - cp]
    return np.asarray(s, dtype=np.float32)


def _round_mant(x, bits):
    """Round fp32 values to `bits` explicit mantissa bits (RNE)."""
    x = np.asarray(x, np.float32)
    b = x.view(np.uint32).copy()
    drop = 23 - bits
    b = b + (((b >> drop) & 1) + np.uint32((1 << (drop - 1)) - 1))
    b &= np.uint32(~((1 << drop) - 1) & 0xFFFFFFFF)
    return b.view(np.float32)


def _round10(x):
    return _round_mant(x, 10)


def _bf16():
    import ml_dtypes

    return ml_dtypes.bfloat16


def _plan(attn_mask):
    """Compaction plan: per-batch unmasked positions, per-tile real-key
    spans, per-(slot,chunk) key-tile bands, per-(tile,chunk) block class
    ('L' pure-left / 'R' pure-right / 'S' straddle), straddle list."""
    attn_mask = np.asarray(attn_mask)
    pos = [np.nonzero(attn_mask[b])[0].astype(np.int64) for b in range(B)]
    for b in range(B):
        assert len(pos[b]) <= NK, f"batch {b} has {len(pos[b])} unmasked keys > {NK}"
    spans = []
    for b in range(B):
        s = []
        for t in range(KT):
            seg = pos[b][t * 128 : (t + 1) * 128]
            s.append((int(seg[0]), int(seg[-1])) if len(seg) else None)
        spans.append(s)
    bands = []
    for sl in range(NH):
        row = []
        for c in range(QC):
            d = D_SLOT[sl]
            if d is None:
                row.append((0, KT))
                continue
            lo_q, hi_q = 512 * c, 512 * c + 511
            tl, th = KT, -1
            for b in range(B):
                for t in range(KT):
                    sp = spans[b][t]
                    if sp is None:
                        continue
                    if sp[1] >= lo_q - d and sp[0] <= hi_q + d:
                        tl = min(tl, t)
                        th = max(th, t)
            assert th >= tl
            row.append((tl, th + 1))
        bands.append(row)
    cls = {}
    for c in range(QC):
        for t in range(KT):
            lpure = all(
                spans[b][t] is None or spans[b][t][0] >= 512 * c + 512 for b in range(B)
            )
            rpure = all(
                spans[b][t] is None or spans[b][t][1] < 512 * c for b in range(B)
            )
            cls[(t, c)] = "L" if lpure else ("R" if rpure else "S")
    straddles = sorted(k for k, v in cls.items() if v == "S")
    return pos, spans, bands, cls, straddles


_PROGRAM_CACHE = {}


def _build_program(bands, cls, straddles):
    key = (tuple(map(tuple, bands)), tuple(sorted(cls.items())), tuple(straddles))
    if key in _PROGRAM_CACHE:
        return _PROGRAM_CACHE[key]

    _ensure_concourse()
    import concourse.mybir as mybir
    import concourse.tile as tile
    from concourse import bacc
    from concourse.bass import ts

    f32 = mybir.dt.float32
    f32r = mybir.dt.float32r
    bf16 = mybir.dt.bfloat16
    fp16 = mybir.dt.float16
    Exp = mybir.ActivationFunctionType.Exp
    Ln = mybir.ActivationFunctionType.Ln
    MULT = mybir.AluOpType.mult
    ADD = mybir.AluOpType.add

    NS = len(straddles)
    sidx = {tc: i for i, tc in enumerate(straddles)}

    nc = bacc.Bacc(None)

    xT_d = nc.dram_tensor("xT", [D, L], bf16, kind="ExternalInput")
    xTc_d = nc.dram_tensor("xTc", [D, NK], bf16, kind="ExternalInput")
    wqA_d = nc.dram_tensor("wqA", [D, 128], bf16, kind="ExternalInput")
    wqB_d = nc.dram_tensor("wqB", [D, 64], bf16, kind="ExternalInput")
    wkA_d = nc.dram_tensor("wkA", [D, 128], bf16, kind="ExternalInput")
    wkB_d = nc.dram_tensor("wkB", [D, 64], bf16, kind="ExternalInput")
    wv_d = nc.dram_tensor("wv", [D, DH * NH], bf16, kind="ExternalInput")
    bias_d = nc.dram_tensor("biasq", [128, 4], f32, kind="ExternalInput")
    bv_d = nc.dram_tensor("bv", [1, DH * NH], bf16, kind="ExternalInput")
    augk_d = nc.dram_tensor("augk", [NH, 5, NK], f32r, kind="ExternalInput")
    augqR_d = nc.dram_tensor("augqR", [NH, 5, L], f32r, kind="ExternalInput")
    augqL_d = nc.dram_tensor("augqL", [NH, 5, L], f32r, kind="ExternalInput")
    expfix_d = nc.dram_tensor(
        "expfix", [128, max(NS, 1) * NH, 512], fp16, kind="ExternalInput"
    )
    woutp_d = nc.dram_tensor("woutp", [256, D], bf16, kind="ExternalInput")
    y_d = nc.dram_tensor("ypart", [L, D], f32, kind="ExternalOutput")

    def band_groups(j, c):
        """Group the band's tiles GROUP_SIZE at a time.  Straddle tiles
        last within each group: their PV must additionally wait for the
        GPSIMD expfix multiply, so lead with the aug-path tiles whose PV
        can start right after exp."""
        t_lo, t_hi = bands[j][c]
        stra = [t for t in range(t_lo, t_hi) if cls[(t, c)] == "S"]
        rest = [t for t in range(t_lo, t_hi) if cls[(t, c)] != "S"]
        order = []
        for i, r in enumerate(rest):
            order.append(r)
            if i < len(stra):
                order.append(stra[i])
        order += stra[len(rest) :]
        return [order[i : i + GROUP_SIZE] for i in range(0, len(order), GROUP_SIZE)]

    with tile.TileContext(nc) as tc:
        with tc.tile_pool(name="persist", bufs=1) as pp:
            bias_sb = pp.tile([128, 4], f32)
            woutp_sb = pp.tile([128, 2, D], bf16)
            expfix_sb = pp.tile([128, max(NS, 1) * NH, 512], fp16)
            onesv_f = pp.tile([128, KT * NH], f32)
            V_sb = pp.tile([128, KT, NH, DH + 1], bf16)
            ctxA = pp.tile([128, L], bf16)
            ctxB = pp.tile([64, L], bf16)
            denom_sb = pp.tile([1, NH * QC * 512], f32)
            ones_bf = pp.tile([1, 128], bf16)
            Kbuf = [
                pp.tile([69, NK], f32r, tag=f"kb{j}", name=f"kb{j}") for j in range(NH)
            ]
            QbufR = [
                pp.tile([69, L], f32r, tag=f"qr{j}", name=f"qr{j}") for j in range(NH)
            ]
            QbufL = [
                pp.tile([69, L], f32r, tag=f"ql{j}", name=f"ql{j}") for j in range(NH)
            ]

            with (
                tc.tile_pool(name="xpool", bufs=1) as xp,
                tc.tile_pool(name="ps1", bufs=2, space="PSUM") as ps1,
                tc.tile_pool(name="psS", bufs=2, space="PSUM") as psS,
                tc.tile_pool(name="po", bufs=2, space="PSUM") as po,
                tc.tile_pool(name="ptp", bufs=4) as ptp,
                tc.tile_pool(name="rp", bufs=2) as rp,
                tc.tile_pool(name="yp", bufs=3) as yp,
            ):
                wkA_sb = xp.tile([128, 6, 128], bf16)
                wkB_sb = xp.tile([128, 6, 64], bf16)
                wqA_sb = xp.tile([128, 6, 128], bf16)
                wqB_sb = xp.tile([128, 6, 64], bf16)
                wv_sb = xp.tile([128, 6, DH * NH], bf16)
                bv_sb = xp.tile([1, DH * NH], bf16)
                xTc_sb = xp.tile([128, 6, NK], bf16)
                xT_sb = xp.tile([128, 6, L], bf16)

                # ---- input DMAs, spread over engine queues, ordered by
                # first use.  sync: K-side, then QbufL copies, then half
                # the y stores; scalar: Q-side; vector: aug rows + small;
                # gpsimd: expfix + woutp.
                nc.sync.dma_start(wkA_sb[:], wkA_d.rearrange("(o p) m -> p o m", p=128))
                nc.sync.dma_start(wkB_sb[:], wkB_d.rearrange("(o p) m -> p o m", p=128))
                nc.sync.dma_start(wv_sb[:], wv_d.rearrange("(o p) m -> p o m", p=128))
                for kt in range(6):
                    nc.sync.dma_start(
                        xTc_sb[:, kt, :],
                        xTc_d.rearrange("(o p) f -> p o f", p=128)[:, kt, :],
                    )
                nc.scalar.dma_start(
                    wqA_sb[:], wqA_d.rearrange("(o p) m -> p o m", p=128)
                )
                nc.scalar.dma_start(
                    wqB_sb[:], wqB_d.rearrange("(o p) m -> p o m", p=128)
                )
                for kt in range(6):
                    nc.scalar.dma_start(
                        xT_sb[:, kt, :],
                        xT_d.rearrange("(o p) f -> p o f", p=128)[:, kt, :],
                    )
                nc.gpsimd.dma_start(bias_sb[:], bias_d[:])
                nc.gpsimd.dma_start(bv_sb[:], bv_d[:])
                for j in range(NH):
                    nc.gpsimd.dma_start(Kbuf[j][64:69, :], augk_d[j])
                    nc.gpsimd.dma_start(QbufR[j][64:69, :], augqR_d[j])
                    nc.gpsimd.dma_start(QbufL[j][64:69, :], augqL_d[j])
                nc.gpsimd.dma_start(expfix_sb[:], expfix_d[:])
                nc.gpsimd.dma_start(
                    woutp_sb[:], woutp_d.rearrange("(o p) n -> p o n", p=128)
                )

                # ones + V denominator column
                nc.gpsimd.memset(onesv_f[:], 1.0)
                onesf = pp.tile([1, 128], f32)
                nc.gpsimd.memset(onesf[:], 1.0)
                nc.vector.tensor_copy(ones_bf[:], onesf[:])
                nc.vector.tensor_copy(
                    V_sb[:, :, :, DH : DH + 1].rearrange("p t h o -> p (t h o)"),
                    onesv_f[:],
                )

                # Pre-touch the DMA-loaded bias so the TensorScalarPtr Q/K
                # stashes carry a single sync wait (the walrus TS encoding
                # rejects multi-wait instructions).
                junkv = pp.tile([1, 4], f32, name="junkv")
                nc.vector.tensor_copy(junkv[0:1, 0:1], bias_sb[0:1, 0:1])

                # ---- stage 1a: K projection (compacted keys) ----
                KC = [(0, 512), (512, 512), (1024, 128)]
                for k0, kw in KC:
                    psA = ps1.tile([128, 512], f32, tag="ps1", name="psKA")[:, :kw]
                    for kt in range(6):
                        nc.tensor.matmul(
                            psA,
                            wkA_sb[:, kt, :],
                            xTc_sb[:, kt, k0 : k0 + kw],
                            start=(kt == 0),
                            stop=(kt == 5),
                        )
                    nc.vector.tensor_scalar(
                        Kbuf[0][0:64, k0 : k0 + kw],
                        psA[0:64, :],
                        bias_sb[0:64, 2:3],
                        None,
                        ADD,
                    )
                    nc.vector.tensor_scalar(
                        Kbuf[1][0:64, k0 : k0 + kw],
                        psA[64:128, :],
                        bias_sb[64:128, 2:3],
                        None,
                        ADD,
                    )
                    psB = ps1.tile([128, 512], f32, tag="ps1", name="psKB")[0:64, :kw]
                    for kt in range(6):
                        nc.tensor.matmul(
                            psB,
                            wkB_sb[:, kt, :],
                            xTc_sb[:, kt, k0 : k0 + kw],
                            start=(kt == 0),
                            stop=(kt == 5),
                        )
                    nc.vector.tensor_scalar(
                        Kbuf[2][0:64, k0 : k0 + kw],
                        psB,
                        bias_sb[0:64, 3:4],
                        None,
                        ADD,
                    )

                def emit_q_chunk(c):
                    cs = ts(c, 512)
                    psA = ps1.tile([128, 512], f32, tag="ps1", name="psQA")
                    for kt in range(6):
                        nc.tensor.matmul(
                            psA,
                            wqA_sb[:, kt, :],
                            xT_sb[:, kt, cs],
                            start=(kt == 0),
                            stop=(kt == 5),
                        )
                    nc.vector.tensor_scalar(
                        QbufR[0][0:64, cs], psA[0:64, :], 0.125,
                        bias_sb[0:64, 0:1], MULT, ADD,
                    )
                    nc.vector.tensor_scalar(
                        QbufR[1][0:64, cs], psA[64:128, :], 0.125,
                        bias_sb[64:128, 0:1], MULT, ADD,
                    )
                    psB = ps1.tile([128, 512], f32, tag="ps1", name="psQB")[0:64, :]
                    for kt in range(6):
                        nc.tensor.matmul(
                            psB,
                            wqB_sb[:, kt, :],
                            xT_sb[:, kt, cs],
                            start=(kt == 0),
                            stop=(kt == 5),
                        )
                    nc.vector.tensor_scalar(
                        QbufR[2][0:64, cs], psB, 0.125,
                        bias_sb[0:64, 1:2], MULT, ADD,
                    )
                    # Q rows are identical between R and L bufs; only the
                    # aug rows differ.  Copy via DMA on the sync queue
                    # (frees DVE; f32r bits already rounded by the TS).
                    for j in range(NH):
                        nc.sync.dma_start(QbufL[j][0:64, cs], QbufR[j][0:64, cs])

                emit_q_chunk(0)

                # ---- stage 1b: V (natural layout, compacted keys) ----
                for lt in range(KT):
                    psv = ps1.tile([128, 512], f32, tag="ps1", name="psv")[
                        :, : DH * NH
                    ]
                    for kt in range(6):
                        nc.tensor.matmul(
                            psv,
                            xTc_sb[:, kt, ts(lt, 128)],
                            wv_sb[:, kt, :],
                            start=(kt == 0),
                            stop=False,
                        )
                    nc.tensor.matmul(
                        psv, ones_bf[0:1, 0:128], bv_sb[0:1, :],
                        start=False, stop=True,
                    )
                    nc.vector.tensor_copy(
                        V_sb[:, lt, :, 0:DH],
                        psv.rearrange("p (h x) -> p h x", x=DH),
                    )

                # ---- stage 2: attention; remaining Q chunks interleaved
                # so exp starts early and the PE never starves ----
                def emit_sgroup(j, c, tlist, st):
                    cs = ts(c, 512)
                    for i, t in enumerate(tlist):
                        js = ts(i, 512)
                        k = cls[(t, c)]
                        if k == "S":
                            # mask-free S = Q.K; ALiBi+mask applied
                            # multiplicatively to pt after exp
                            nc.tensor.matmul(
                                st[:, js],
                                Kbuf[j][0:64, ts(t, 128)],
                                QbufR[j][0:64, cs],
                                start=True,
                                stop=True,
                            )
                        else:
                            qb = QbufL[j] if k == "L" else QbufR[j]
                            nc.tensor.matmul(
                                st[:, js],
                                Kbuf[j][0:69, ts(t, 128)],
                                qb[0:69, cs],
                                start=True,
                                stop=True,
                            )

                def emit_exp_pv(j, c, tlist, st, out_t, first, last):
                    tn = len(tlist)
                    pt = ptp.tile([128, GROUP_SIZE * 512], bf16, tag="pt", name="pt")
                    nc.scalar.activation(pt[:, : tn * 512], st[:, : tn * 512], Exp)
                    for i, t in enumerate(tlist):
                        if cls[(t, c)] == "S":
                            fi = sidx[(t, c)] * NH + j
                            nc.gpsimd.tensor_mul(
                                pt[:, ts(i, 512)],
                                pt[:, ts(i, 512)],
                                expfix_sb[:, fi, :],
                            )
                    for i, t in enumerate(tlist):
                        nc.tensor.matmul(
                            out_t[0 : DH + 1, :],
                            V_sb[:, t, j, :],
                            pt[:, ts(i, 512)],
                            start=(first and i == 0),
                            stop=(last and i == tn - 1),
                            skip_group_check=True,
                        )

                def emit_pair(c, j):
                    cs = ts(c, 512)
                    out_t = po.tile([128, 512], f32, tag="po", name="outaug")
                    gl = band_groups(j, c)
                    ng = len(gl)
                    sts = []
                    for g, tlist in enumerate(gl):
                        st = psS.tile(
                            [128, GROUP_SIZE * 512], f32, tag="st", name="st"
                        )
                        sts.append(st)
                        emit_sgroup(j, c, tlist, st)
                        if g >= 1:
                            emit_exp_pv(
                                j, c, gl[g - 1], sts[g - 1], out_t,
                                g - 1 == 0, False,
                            )
                    emit_exp_pv(j, c, gl[-1], sts[-1], out_t, ng == 1, True)
                    # stash unnormalized ctx + denom row (both DVE; GPSIMD
                    # cannot read PSUM)
                    if j < 2:
                        ctx_slice = ctxA[j * 64 : (j + 1) * 64, cs]
                    else:
                        ctx_slice = ctxB[0:64, cs]
                    nc.vector.tensor_copy(ctx_slice, out_t[0:DH, :])
                    nc.vector.tensor_copy(
                        denom_sb[0:1, ts(c * NH + j, 512)],
                        out_t[DH : DH + 1, :],
                    )

                for c in range(QC):
                    for j in range(NH):
                        emit_pair(c, j)
                        if j == 0 and c + 1 < QC:
                            emit_q_chunk(c + 1)

                # ---- stage 2b: denominators -> reciprocals.  One Ln +
                # one Exp(-x) after all stage-2 exps: the Ln table load
                # and the Exp reload are the only two switches. ----
                nc.scalar.activation(denom_sb[:], denom_sb[:], Ln)
                nc.scalar.activation(denom_sb[:], denom_sb[:], Exp, scale=-1.0)

                # ---- stage 3: normalize + output projection per chunk ----
                for c in range(QC):
                    cs = ts(c, 512)
                    for j in range(NH):
                        recb = rp.tile([128, 512], f32, tag="recb")
                        nc.gpsimd.partition_broadcast(
                            recb, denom_sb[0:1, ts(c * NH + j, 512)]
                        )
                        if j < 2:
                            ctx_slice = ctxA[j * 64 : (j + 1) * 64, cs]
                            recs = recb[j * 64 : (j + 1) * 64, :]
                        else:
                            ctx_slice = ctxB[0:64, cs]
                            recs = recb[0:64, :]
                        nc.gpsimd.tensor_mul(ctx_slice, ctx_slice, recs)
                    for lt in range(4 * c, 4 * c + 4):
                        y = yp.tile([128, D], f32, tag="y")
                        for n0, nw in ((0, 512), (512, 256)):
                            ps = po.tile([128, 512], f32, tag="po", name="ps3t")[
                                :, :nw
                            ]
                            nc.tensor.matmul(
                                ps,
                                ctxA[:, ts(lt, 128)],
                                woutp_sb[:, 0, n0 : n0 + nw],
                                start=True,
                                stop=False,
                            )
                            nc.tensor.matmul(
                                ps,
                                ctxB[0:64, ts(lt, 128)],
                                woutp_sb[0:64, 1, n0 : n0 + nw],
                                start=False,
                                stop=True,
                            )
                            nc.vector.tensor_copy(y[:, n0 : n0 + nw], ps)
                        if lt % 2 == 0:
                            nc.sync.dma_start(y_d[ts(lt, 128), :], y)
                        else:
                            nc.scalar.dma_start(y_d[ts(lt, 128), :], y)

    if not nc.is_finalized():
        nc.finalize()
    _PROGRAM_CACHE[key] = nc
    return nc


def _host_inputs(x, attn_mask, Wqkv, bqkv, Wout, bout, plan):
    pos, spans, bands, cls, straddles = plan
    NS = len(straddles)
    slopes_r = _round10(alibi_slopes(H))
    x = np.asarray(x, dtype=np.float32)
    Wqkv = np.asarray(Wqkv, dtype=np.float32)
    bqkv = np.asarray(bqkv, dtype=np.float32)
    Wout = np.asarray(Wout, dtype=np.float32)

    q_idx = np.arange(L, dtype=np.float64)
    ones_row_L = np.ones(L, dtype=np.float32)
    ones_row_K = np.ones(NK, dtype=np.float32)
    bf = _bf16()

    in_maps = []
    for core in range(N_CORES):
        b = core // 4
        g = core % 4
        heads = HEAD_GROUPS[g]
        pb = pos[b]
        n = len(pb)

        o_arr = np.zeros(NK, dtype=np.float64)
        o_arr[:n] = pb

        xc = np.zeros((NK, D), np.float32)
        xc[:n] = x[b][pb]

        wqA = np.empty((D, 128), np.float32)
        wqB = np.empty((D, 64), np.float32)
        wkA = np.empty((D, 128), np.float32)
        wkB = np.empty((D, 64), np.float32)
        wv = np.empty((D, DH * NH), np.float32)
        bias = np.zeros((128, 4), np.float32)
        bv = np.empty((1, DH * NH), np.float32)
        woutp = np.zeros((256, D), np.float32)
        augk = np.empty((NH, 5, NK), np.float32)
        augqR = np.empty((NH, 5, L), np.float32)
        augqL = np.empty((NH, 5, L), np.float32)
        expfix = np.zeros((128, max(NS, 1) * NH, 512), np.float16)

        for jj, h in enumerate(heads):
            qs = slice(h * DH, (h + 1) * DH)
            ks = slice(D + h * DH, D + (h + 1) * DH)
            vs = slice(2 * D + h * DH, 2 * D + (h + 1) * DH)
            if jj < 2:
                wqA[:, jj * 64 : (jj + 1) * 64] = Wqkv[:, qs]
                wkA[:, jj * 64 : (jj + 1) * 64] = Wqkv[:, ks]
                bias[jj * 64 : (jj + 1) * 64, 0] = bqkv[qs] * 0.125
                bias[jj * 64 : (jj + 1) * 64, 2] = bqkv[ks]
            else:
                wqB[:, 0:64] = Wqkv[:, qs]
                wkB[:, 0:64] = Wqkv[:, ks]
                bias[0:64, 1] = bqkv[qs] * 0.125
                bias[0:64, 3] = bqkv[ks]
            wv[:, jj * DH : (jj + 1) * DH] = Wqkv[:, vs]
            bv[0, jj * DH : (jj + 1) * DH] = bqkv[vs]
            woutp[jj * DH : (jj + 1) * DH, :] = Wout[h * DH : (h + 1) * DH, :]

            s = float(slopes_r[h])
            # s*idx premultiplied and split into 11-bit value + residual
            # so the f32r PE preserves each product (value * +-1) exactly
            sk = np.float64(s) * o_arr
            vk = _round_mant(sk.astype(np.float32), 11)
            rvk = _round_mant((sk - vk.astype(np.float64)).astype(np.float32), 11)
            vk[n:] = 0.0
            rvk[n:] = 0.0
            sq = np.float64(s) * q_idx
            rq = _round_mant(sq.astype(np.float32), 11)
            rr = _round_mant((sq - rq.astype(np.float64)).astype(np.float32), 11)

            maskrow = np.zeros(NK, np.float32)
            maskrow[n:] = NEG_MASK
            augk[jj, 0, :] = _round_mant(maskrow, 11)
            augk[jj, 1, :] = vk
            augk[jj, 2, :] = rvk
            augk[jj, 3, :] = ones_row_K
            augk[jj, 4, :] = ones_row_K
            augqR[jj, 0, :] = ones_row_L
            augqR[jj, 1, :] = ones_row_L
            augqR[jj, 2, :] = ones_row_L
            augqR[jj, 3, :] = -rq
            augqR[jj, 4, :] = -rr
            augqL[jj, 0, :] = ones_row_L
            augqL[jj, 1, :] = -ones_row_L
            augqL[jj, 2, :] = -ones_row_L
            augqL[jj, 3, :] = rq
            augqL[jj, 4, :] = rr

            # post-exp ALiBi factors for straddle blocks: exp(-s|q-o(k)|)
            # (fp16; 0 at padding keys, which also removes them from the
            # softmax without a mask row)
            for i, (t, c) in enumerate(straddles):
                nreal = max(0, min(n - t * 128, 128))
                if nreal <= 0:
                    continue
                seg = o_arr[t * 128 : t * 128 + nreal]
                qv = 512 * c + np.arange(512, dtype=np.float64)[None, :]
                f = np.exp(-np.float64(s) * np.abs(qv - seg[:, None]))
                expfix[:nreal, i * NH + jj, :] = f.astype(np.float16)

        in_maps.append(
            {
                "xT": np.ascontiguousarray(x[b].T).astype(bf),
                "xTc": np.ascontiguousarray(xc.T).astype(bf),
                "wqA": wqA.astype(bf),
                "wqB": wqB.astype(bf),
                "wkA": wkA.astype(bf),
                "wkB": wkB.astype(bf),
                "wv": wv.astype(bf),
                "biasq": bias,
                "bv": bv.astype(bf),
                "augk": augk.copy(),
                "augqR": augqR.copy(),
                "augqL": augqL.copy(),
                "expfix": expfix,
                "woutp": woutp.astype(bf),
            }
        )
    return in_maps


def kernel(x, attn_mask, Wqkv, bqkv, Wout, bout):
    _ensure_concourse()
    from concourse.bass_utils import run_bass_kernel_spmd

    plan = _plan(attn_mask)
    nc = _build_program(plan[2], plan[3], plan[4])
    in_maps = _host_inputs(x, attn_mask, Wqkv, bqkv, Wout, bout, plan)

    res = run_bass_kernel_spmd(
        nc,
        in_maps,
        list(range(N_CORES)),
        trace=bool(os.environ.get("BASS_TRACE")),
    )
    outs = [r["ypart"] for r in res.results]
    out = np.zeros((B, L, D), np.float32)
    for core in range(N_CORES):
        out[core // 4] += outs[core]
    out += np.asarray(bout, np.float32)[None, None, :]
    kernel.last_result = res
    if res.exec_time_ns is not None:
        kernel.last_exec_time_ns = res.exec_time_ns
    return out
